# revision 36
# baseline (speedup 1.0000x reference)
"""GQA attention kernel for 8 Trainium2 NeuronCores (Bass/Tile).

Sharding: data-parallel over batch (2) x tensor-parallel over head groups (4).
Core c: batch b=c//4, group g=c%4 (query heads 4g..4g+3, kv head g).
w_q/w_k/w_v column-parallel, w_o row-parallel; partial outputs are
ReduceScattered (bf16) on-device over groups [[0..3],[4..7]]; host gather is a
pure concatenation + fp32 cast.

Hardcoded problem: B=2 T=2048 D=1024 n_heads=16 n_kv=4 d_head=64, causal,
RoPE theta=1e4 (freqs passed as input), scale=1/8.

Perf structure:
- QK^T packed 2 heads/slot via PE row-tiling (contraction=64): kT duplicated
  on partitions 0-63/64-127, q head pairs on matching halves.
- exp batched per head-pair: one ACT instruction over [128, 2, 512-o0].
- causal diag mask: one gpsimd affine_select per (pair, diag block).
- scores psum [128,2,512] double-buffered (4 banks), oa accumulators merged
  [128,4,65] (1 bank each, 2 bufs), qkv 1 bank, transposes 1 bank.
- ReduceScatter in bf16 (half payload); out tensor bf16, cast on host.
"""

import numpy as np

import concourse.bass as bass
import concourse.tile as tile
from concourse import bacc, mybir
from concourse.bass_utils import run_bass_kernel_spmd
from concourse.masks import make_identity

F32 = mybir.dt.float32
BF16 = mybir.dt.bfloat16

B, T, D = 2, 2048, 1024
NH, NKV, DH = 16, 4, 64
HPC = NH // NKV          # query heads per core = 4
OC = HPC * DH            # per-core attn feature cols = 256
TB = T // 128            # 16 blocks of 128 rows
NJ = T // 512            # 4 tq-slices of 512
GROUPS = [[0, 1, 2, 3], [4, 5, 6, 7]]
SCALE = 1.0 / 8.0

_CACHE = {}


def _emit(nc, tc, aps):
    x_ap, wq_ap, wk_ap, wv_ap, wo_ap, rope_ap, out_ap = aps
    import contextlib
    ctx = contextlib.ExitStack()
    with ctx:
        sing = ctx.enter_context(tc.tile_pool(name="sing", bufs=1))
        stage = ctx.enter_context(tc.tile_pool(name="stage", bufs=3))
        bstage = ctx.enter_context(tc.tile_pool(name="bstage", bufs=3))
        ropet = ctx.enter_context(tc.tile_pool(name="ropet", bufs=6))
        qrp = ctx.enter_context(tc.tile_pool(name="qrp", bufs=2))
        ptp = ctx.enter_context(tc.tile_pool(name="ptp", bufs=3))
        onatp = ctx.enter_context(tc.tile_pool(name="onatp", bufs=8))
        outsbp = ctx.enter_context(tc.tile_pool(name="outsbp", bufs=3))
        rcp = ctx.enter_context(tc.tile_pool(name="rcp", bufs=8))
        # PSUM pools (slots are per-tag x bufs):
        # scp: tag st [128,2,512]f32 = 2 banks x2 = 4 (also hosts qkv proj,
        #      the bcast and O-proj matmul outputs as subviews)
        # oap: tag oa [128,512]f32 1 bank x2 = 2 (col-packed pair (AV)^T)
        # rsp: tag rs [128,512]f32 1 bank x1 (rowsums, partitions {0,32,64,96})
        # trstp: tag trst [128,512]bf16 1 bank x1
        scp = ctx.enter_context(tc.tile_pool(name="scp", bufs=2, space="PSUM"))
        oap = ctx.enter_context(tc.tile_pool(name="oap", bufs=2, space="PSUM"))
        rsp = ctx.enter_context(tc.tile_pool(name="rsp", bufs=1, space="PSUM"))
        trstp = ctx.enter_context(tc.tile_pool(name="trstp", bufs=1, space="PSUM"))
        dram = ctx.enter_context(tc.tile_pool(name="dram", bufs=1, space="DRAM"))

        # ---- warm-up collective: pays the rendezvous cost concurrently
        d_in = dram.tile([1, 64], F32)
        d_out = dram.tile([1, 64], F32)
        zt = sing.tile([1, 64], F32)
        nc.vector.memset(zt[:], 0.0)
        nc.sync.dma_start(d_in[:], zt[:])
        nc.gpsimd.collective_compute(
            "AllReduce", mybir.AluOpType.add, replica_groups=GROUPS,
            ins=[d_in.opt()], outs=[d_out.opt()])

        # ---- persistent SBUF tensors
        identb = sing.tile([128, 128], BF16)
        make_identity(nc, identb[:])
        xT = sing.tile([128, 8, T], BF16)        # [d-chunk part, chunk, t]
        wT = sing.tile([128, 8, 384], BF16)      # cols: 0:256 wq | 256:320 wk | 320:384 wv
        woT = sing.tile([128, 2, D], BF16)       # [o-chunk part, chunk, dout]
        # qkT: slot 0 = heads (0,1), slot 1 = heads (2,3), slot 2 = K dup both halves
        qkT = sing.tile([128, 3, T], BF16)
        vfl = sing.tile([128, TB, 64], BF16)     # V blocks [t-part, tb, d]
        ones1 = sing.tile([128, 1], BF16)        # rowsum stationary
        oT = sing.tile([128, 2, T], BF16)
        rope_sb = sing.tile([128, TB, 5, 64], F32)
        nc.vector.memset(ones1[:], 1.0)
        # causal mask for diagonal blocks: trimask[p, f] = 1 if f >= p else 0
        trimask = sing.tile([128, 128], BF16)
        nc.vector.memset(trimask[:], 1.0)
        nc.gpsimd.affine_select(
            out=trimask[:], in_=trimask[:], compare_op=mybir.AluOpType.is_ge,
            fill=0.0, base=0, pattern=[[1, 128]], channel_multiplier=-1)
        # SEL[r, p, p'] selector: bcast rcT row 64p+32*(p'//64) to out row p'
        sel = sing.tile([128, 2, 128], BF16)
        nc.vector.memset(sel[:], 0.0)
        for _p in range(2):
            for _s in range(2):
                nc.vector.memset(sel[64 * _p + 32 * _s:64 * _p + 32 * _s + 1,
                                     _p, 64 * _s:64 * _s + 64], 1.0)
        # norm-chain scratch, one slot per head pair (reused every j; rcs
        # pre-zeroed so junk columns transpose to exact zeros for the
        # selector matmul)
        rssb = sing.tile([128, 2, 512], BF16)
        nc.vector.memset(rssb[:], 1.0)
        trsb = sing.tile([128, 2, 256], BF16)
        rcs = sing.tile([128, 2, 256], BF16)
        nc.vector.memset(rcs[:], 0.0)
        rcT = sing.tile([128, 512], BF16)
        nc.vector.memset(rcT[:], 0.0)
        bcsb = sing.tile([128, 2, 512], BF16)
        _r = rope_ap.rearrange("(tb p) f -> p tb f", p=128)
        for _tb in range(TB):
            _rt = _r[:, _tb, :]
            _r5 = bass.AP(tensor=_rt.tensor, offset=_rt.offset,
                          ap=[_rt.ap[0], [0, 5], _rt.ap[1]])
            nc.sync.dma_start(rope_sb[:, _tb, :, :], _r5)

        # ---- weights: load, cast to bf16, transpose (batched copies)
        for r in range(2):  # wq rows 256 -> 2 tiles of 128
            wn = stage.tile([128, 1024], F32, tag="wstage")
            nc.sync.dma_start(wn[:], wq_ap[128 * r:128 * (r + 1), :])
            wb = bstage.tile([128, 1024], BF16, tag="wbst")
            nc.vector.tensor_copy(wb[:], wn[:])
            for g in range(2):  # 4 chunk-transposes per trst tile
                tr = trstp.tile([128, 512], BF16, tag="trst")
                for k in range(4):
                    dch = 4 * g + k
                    nc.tensor.transpose(tr[:, 128 * k:128 * (k + 1)],
                                        wb[:, 128 * dch:128 * (dch + 1)], identb[:])
                dst = wT[:, 4 * g:4 * g + 4, 128 * r:128 * (r + 1)]
                nc.vector.tensor_copy(dst, tr[:].rearrange("p (k f) -> p k f", k=4))
        for w_ap, col0 in ((wk_ap, 256), (wv_ap, 320)):
            wn = stage.tile([128, 1024], F32, tag="wstage")
            nc.sync.dma_start(wn[:64, :], w_ap[:, :])
            wb = bstage.tile([128, 1024], BF16, tag="wbst")
            nc.vector.tensor_copy(wb[:64, :], wn[:64, :])
            tr = trstp.tile([128, 512], BF16, tag="trst")
            for dch in range(8):
                nc.tensor.transpose(tr[:, 64 * dch:64 * (dch + 1)],
                                    wb[:64, 128 * dch:128 * (dch + 1)],
                                    identb[:64, :64])
            dst = wT[:, :, col0:col0 + 64]
            nc.vector.tensor_copy(dst, tr[:].rearrange("p (k f) -> p k f", k=8))
        for r in range(8):  # wo (1024, 256) -> 8 row tiles
            wn = stage.tile([128, 256], F32, tag="wostage")
            nc.sync.dma_start(wn[:], wo_ap[128 * r:128 * (r + 1), :])
            wb = bstage.tile([128, 256], BF16, tag="wobst")
            nc.vector.tensor_copy(wb[:], wn[:])
            tr = trstp.tile([128, 512], BF16, tag="trst")
            for oc in range(2):
                nc.tensor.transpose(tr[:, 128 * oc:128 * (oc + 1)],
                                    wb[:, 128 * oc:128 * (oc + 1)], identb[:])
            dst = woT[:, :, 128 * r:128 * (r + 1)]
            nc.vector.tensor_copy(dst, tr[:, 0:256].rearrange("p (k f) -> p k f", k=2))

        # ---- per t-block phase12, split into pipelined chunks:
        # chunk A: x load/cast, 8 transposes into ONE [128,1024]bf16 psum bank,
        #          one copy, QKV proj, rope staging copy + rope (DVE)
        # chunk B (emitted one block later): Q/K transposes + copy
        qrs = {}

        def p12a(tb):
            xs = stage.tile([128, 1024], F32, tag="xstage")
            nc.sync.dma_start(xs[:], x_ap[128 * tb:128 * (tb + 1), :])
            xb = bstage.tile([128, 1024], BF16, tag="xbst")
            nc.vector.tensor_copy(xb[:], xs[:])
            tr = trstp.tile([128, 1024], BF16, tag="trst")
            for dch in range(8):
                nc.tensor.transpose(tr[:, 128 * dch:128 * (dch + 1)],
                                    xb[:, 128 * dch:128 * (dch + 1)], identb[:])
            dst = xT[:, :, 128 * tb:128 * (tb + 1)]
            nc.vector.tensor_copy(dst, tr[:].rearrange("p (k f) -> p k f", k=8))
            qkvt = scp.tile([128, 2, 512], F32, tag="st", name="qkvt")
            qkv = qkvt[:, 0, 0:384]
            for dch in range(8):
                nc.tensor.matmul(qkv, xT[:, dch, 128 * tb:128 * (tb + 1)],
                                 wT[:, dch, :], start=(dch == 0), stop=(dch == 7))
            # stage out of psum fast (frees the scp slot), then rope on sbuf
            qksb = ropet.tile([128, 384], F32, tag="qksb")
            nc.vector.tensor_copy(qksb[:], qkv)
            nc.vector.tensor_copy(vfl[:, tb, :], qksb[:, 320:384])
            # rope: tA = qk*[cos,cos]; tB = qk*[sin,sin];
            # out_re = tA_re - tB_im; out_im = tB_re + tA_im
            qk5 = qksb[:, 0:320].rearrange("p (g i c) -> p g i c", g=5, c=2)
            rv = rope_sb[:][:, tb, :, :].rearrange("p g (i c) -> p g i c", c=2)
            cos_b, sin_b = rv[:, :, :, 0], rv[:, :, :, 1]
            tA = ropet.tile([128, 5, 32, 2], F32, tag="tA")
            tB = ropet.tile([128, 5, 32, 2], F32, tag="tB")
            ccv = bass.AP(tensor=cos_b.tensor, offset=cos_b.offset,
                          ap=cos_b.ap + [[0, 2]])
            ssv = bass.AP(tensor=sin_b.tensor, offset=sin_b.offset,
                          ap=sin_b.ap + [[0, 2]])
            nc.gpsimd.tensor_mul(tA[:], qk5, ccv)
            nc.gpsimd.tensor_mul(tB[:], qk5, ssv)
            qr = qrp.tile([128, 320], BF16, tag="qr")
            q4 = qr[:].rearrange("p (g i c) -> p g i c", g=5, c=2)
            nc.vector.tensor_sub(q4[:, :, :, 0], tA[:, :, :, 0], tB[:, :, :, 1])
            nc.vector.tensor_add(q4[:, :, :, 1], tB[:, :, :, 0], tA[:, :, :, 1])
            qrs[tb] = qr

        def p12b(tb):
            # pair p: head 2p on partitions 0-63, 2p+1 on 64-127; K duplicated
            qr = qrs.pop(tb)
            tr = trstp.tile([128, 1024], BF16, tag="trst")
            for p in range(2):
                nc.tensor.transpose(tr[0:64, 128 * p:128 * (p + 1)],
                                    qr[:, 128 * p:128 * p + 64], identb[:])
                nc.tensor.transpose(tr[64:128, 128 * p:128 * (p + 1)],
                                    qr[:, 128 * p + 64:128 * (p + 1)], identb[:])
            nc.tensor.transpose(tr[0:64, 256:384], qr[:, 256:320], identb[:])
            nc.tensor.transpose(tr[64:128, 256:384], qr[:, 256:320], identb[:])
            nc.vector.tensor_copy(qkT[:, :, 128 * tb:128 * (tb + 1)],
                                  tr[:, 0:384].rearrange("p (s f) -> p s f", s=3))

        partial = dram.tile([T, D], BF16)

        # ---- attention for tq-slice j, one head pair p. Software-pipelined:
        # AV/rowsum for block i are emitted after QK/exp of block i+1 so the
        # in-order PE never stalls on the ACT exp of the current block.
        def phase3_att(j, p, oaT, rs, filler):
            last = 4 * j + 3
            pts = {}

            def qk_exp(i):
                o0 = max(0, 128 * i - 512 * j)
                st = scp.tile([128, 2, 512], F32, tag="st")
                nc.tensor.matmul(
                    st[:, 0, o0:512],
                    qkT[0:64, 2, 128 * i:128 * (i + 1)],
                    qkT[0:64, p, 512 * j + o0:512 * (j + 1)],
                    start=True, stop=True)
                nc.tensor.matmul(
                    st[:, 1, o0:512],
                    qkT[64:128, 2, 128 * i:128 * (i + 1)],
                    qkT[64:128, p, 512 * j + o0:512 * (j + 1)],
                    start=True, stop=True)
                pt = ptp.tile([128, 2, 512], BF16, tag="pt")
                if o0 == 0:
                    nc.scalar.activation(pt[:].rearrange("a b c -> a (b c)"),
                                         st[:].rearrange("a b c -> a (b c)"),
                                         mybir.ActivationFunctionType.Exp,
                                         scale=SCALE)
                else:
                    nc.scalar.activation(pt[:, :, o0:512], st[:, :, o0:512],
                                         mybir.ActivationFunctionType.Exp,
                                         scale=SCALE)
                if i >= 4 * j:  # diagonal block: zero tq < tk after exp
                    c = i - 4 * j
                    tm = trimask[:]
                    tm3 = bass.AP(tensor=tm.tensor, offset=tm.offset,
                                  ap=[tm.ap[0], [0, 2], tm.ap[1]])
                    nc.vector.tensor_mul(pt[:, :, 128 * c:128 * (c + 1)],
                                         pt[:, :, 128 * c:128 * (c + 1)], tm3)
                pts[i] = pt

            def av(i):
                o0 = max(0, 128 * i - 512 * j)
                pt = pts.pop(i)
                # (AV)^T col-packed: head 2p -> partitions 0-63, 2p+1 -> 64-127
                nc.tensor.matmul(oaT[0:64, o0:512], vfl[:, i, :],
                                 pt[:, 0, o0:512],
                                 start=(i == 0), stop=(i == last),
                                 skip_group_check=True)
                nc.tensor.matmul(oaT[64:128, o0:512], vfl[:, i, :],
                                 pt[:, 1, o0:512],
                                 start=(i == 0), stop=(i == last),
                                 skip_group_check=True)
                # rowsums (ones stationary) at partitions 0 / 32
                for s in range(2):
                    nc.tensor.matmul(rs[32 * s:32 * s + 1, o0:512], ones1[:],
                                     pt[:, s, o0:512],
                                     start=(i == 0), stop=(i == last),
                                     skip_group_check=True)

            for i in range(4 * j + 4):
                qk_exp(i)
                if i >= 1:
                    av(i - 1)
                filler()
            av(4 * j + 3)

        # normalization chain for pair p: rs -> sbuf (norm_a, frees the rs
        # bank early) -> transpose -> recip -> transpose back -> selector-
        # matmul broadcast -> multiply into oT (norm_b)
        def norm_a(p, rs):
            nc.vector.tensor_copy(rssb[0:1, p, :], rs[0:1, :])
            nc.vector.tensor_copy(rssb[32:33, p, :], rs[32:33, :])

        def norm_b(j, p, oaT):
            pb = 64 * p
            tr1 = trstp.tile([128, 1024], BF16, tag="trst", name="tr1")
            for c in range(4):
                nc.tensor.transpose(tr1[:, 64 * c:64 * (c + 1)],
                                    rssb[0:64, p, 128 * c:128 * (c + 1)],
                                    identb[:64, :64])
            nc.vector.tensor_copy(trsb[:, p, :], tr1[:, 0:256])
            tv = trsb[:, p, :].rearrange("q (c s r) -> q c s r", c=4, s=2)
            rv2 = rcs[:, p, :].rearrange("q (c s r) -> q c s r", c=4, s=2)
            with nc.allow_low_precision(reason="softmax denom recip in bf16"):
                nc.vector.reciprocal(rv2[:, :, :, 0], tv[:, :, :, 0])
            tr2 = trstp.tile([128, 1024], BF16, tag="trst", name="tr2")
            for c in range(4):
                nc.tensor.transpose(tr2[pb:pb + 64, 128 * c:128 * (c + 1)],
                                    rcs[:, p, 64 * c:64 * (c + 1)], identb[:])
            nc.vector.tensor_copy(rcT[pb:pb + 64, :], tr2[pb:pb + 64, 0:512])
            bcb = scp.tile([128, 2, 512], F32, tag="st", name="bcb")
            nc.tensor.matmul(bcb[:, 0, :], sel[:, p, :], rcT[:], start=True,
                             stop=True)
            nc.vector.tensor_copy(bcsb[:, p, :], bcb[:, 0, :])
            nc.vector.tensor_mul(oT[:, p, 512 * j:512 * (j + 1)], oaT[:],
                                 bcsb[:, p, :])

        # output projection + partial store for tq block tb (alternating the
        # psum->sbuf copies between vector and scalar engines)
        def oproj(tb):
            for ns in range(2):
                opt_ = scp.tile([128, 2, 512], F32, tag="st", name=f"op{ns}")
                op = opt_[:, 0, :]
                for oc in range(2):
                    nc.tensor.matmul(op, oT[:, oc, 128 * tb:128 * (tb + 1)],
                                     woT[:, oc, 512 * ns:512 * (ns + 1)],
                                     start=(oc == 0), stop=(oc == 1))
                ob = outsbp.tile([128, 512], BF16, tag="outsb")
                nc.vector.tensor_copy(ob[:], op)
                nc.sync.dma_start(
                    partial[128 * tb:128 * (tb + 1), 512 * ns:512 * (ns + 1)],
                    ob[:])

        def reduce_scatter(j):
            # rows 512j..512j+512 complete -> ReduceScatter this quarter (bf16)
            rsout = dram.tile([128, D], BF16, name=f"rsout{j}")
            nc.gpsimd.collective_compute(
                "ReduceScatter", mybir.AluOpType.add, replica_groups=GROUPS,
                ins=[partial[512 * j:512 * (j + 1), :].opt()],
                outs=[rsout.opt()])
            nc.sync.dma_start(out_ap[128 * j:128 * (j + 1), :], rsout[:])

        # Emission order keeps the in-order PE busy: phase12 of slice j+1 and
        # the output projection of slice j-1 are interleaved as fillers into
        # slice j's attention iterations; norm chains overlap the next pair.
        for tb in range(4):
            p12a(tb)
            if tb >= 1:
                p12b(tb - 1)
        p12b(3)

        for j in range(NJ):
            fillers = []
            if j > 0:  # front-loaded: gets the collective in flight early
                for tb in range(4 * (j - 1), 4 * j):
                    fillers.append(lambda tb=tb: oproj(tb))
                fillers.append(lambda j=j: reduce_scatter(j - 1))
            if j < NJ - 1:
                for tb in range(4 * j + 4, 4 * j + 8):
                    fillers.append(lambda tb=tb: p12a(tb))
                    fillers.append(lambda tb=tb: p12b(tb))
            n_iters = 2 * (4 * j + 4)
            stride = max(1, n_iters // max(1, len(fillers)))
            state = {"it": 0, "fi": 0}

            def filler():
                state["it"] += 1
                while (state["fi"] < len(fillers)
                       and state["it"] >= stride * (state["fi"] + 1)):
                    fillers[state["fi"]]()
                    state["fi"] += 1

            oaTs, rss = [], []
            for p in range(2):
                oaTs.append(oap.tile([128, 512], F32, tag="oa", name=f"oaT{p}"))
                rss.append(rsp.tile([128, 512], F32, tag="rs", name=f"rs{p}"))
                phase3_att(j, p, oaTs[p], rss[p], filler)
                norm_a(p, rss[p])
            while state["fi"] < len(fillers):
                fillers[state["fi"]]()
                state["fi"] += 1
            norm_b(j, 0, oaTs[0])
            norm_b(j, 1, oaTs[1])
        for tb in range(12, 16):
            oproj(tb)
        reduce_scatter(3)


def _build():
    if "nc" in _CACHE:
        return _CACHE["nc"]
    nc = bacc.Bacc("TRN2", target_bir_lowering=False, debug=False, num_devices=8)
    x_ap = nc.dram_tensor("x", [T, D], F32, kind="ExternalInput").ap()
    wq_ap = nc.dram_tensor("wq", [OC, D], F32, kind="ExternalInput").ap()
    wk_ap = nc.dram_tensor("wk", [DH, D], F32, kind="ExternalInput").ap()
    wv_ap = nc.dram_tensor("wv", [DH, D], F32, kind="ExternalInput").ap()
    wo_ap = nc.dram_tensor("wo", [D, OC], F32, kind="ExternalInput").ap()
    rope_ap = nc.dram_tensor("rope", [T, DH], F32, kind="ExternalInput").ap()
    out_ap = nc.dram_tensor("out", [T // 4, D], BF16, kind="ExternalOutput").ap()
    with tile.TileContext(nc) as tc:
        _emit(nc, tc, (x_ap, wq_ap, wk_ap, wv_ap, wo_ap, rope_ap, out_ap))
    nc.compile()
    _CACHE["nc"] = nc
    return nc


def run(trace=False, **inputs):
    x = inputs["x"]
    rope2 = np.ascontiguousarray(
        inputs["rope_freqs"].astype(np.float32).reshape(T, DH))
    w_q, w_k, w_v, w_o = (np.asarray(inputs[k], np.float32)
                          for k in ("w_q", "w_k", "w_v", "w_o"))
    nc = _build()
    in_maps = []
    for c in range(8):
        b, g = divmod(c, 4)
        in_maps.append({
            "x": np.ascontiguousarray(x[b], dtype=np.float32),
            "wq": np.ascontiguousarray(w_q[OC * g:OC * (g + 1)]),
            "wk": np.ascontiguousarray(w_k[DH * g:DH * (g + 1)]),
            "wv": np.ascontiguousarray(w_v[DH * g:DH * (g + 1)]),
            "wo": np.ascontiguousarray(w_o[:, OC * g:OC * (g + 1)]),
            "rope": rope2,
        })
    res = run_bass_kernel_spmd(nc, in_maps, core_ids=list(range(8)), trace=trace)
    out = np.empty((B, T, D), np.float32)
    for core in range(8):
        b, r = divmod(core, 4)
        for c in range(4):
            out[b, 512 * c + 128 * r:512 * c + 128 * (r + 1)] = \
                res.results[core]["out"][128 * c:128 * (c + 1)].astype(np.float32)
    return out, res


def kernel(**inputs):
    out, _ = run(trace=False, **inputs)
    return out
